# revision 1
# baseline (speedup 1.0000x reference)
"""HMM 3-point interpolator render kernel for Trainium2 (8 NeuronCores).

Strategy:
  - Data-parallel over batch B=8: core b renders batch b.
  - Host precomputes, per batch, a per-frame gather index into a
    [3N+1, F] table = concat(start, mid, end, zeros-row). Frames past the
    utterance total point at the zeros row / are skipped entirely.
  - Device kernel: indirect-DMA gather of 128 rows/tile from HBM into
    SBUF, then plain DMA store to the output. Only ceil(max_total/128)
    tiles are rendered; the rest of the output stays at its zero
    initialization (run_bass_kernel_spmd zero-fills ExternalOutputs).
"""

import numpy as np

import concourse.bacc as bacc
import concourse.bass as bass
import concourse.mybir as mybir
import concourse.tile as tile
from concourse.bass_utils import run_bass_kernel_spmd

P = 128


def _build_program(n_rows: int, F: int, ntiles: int):
    """Bass program: gather `ntiles` tiles of 128 rows from table, store to out."""
    nc = bacc.Bacc("TRN2", target_bir_lowering=False, debug=False)

    table = nc.dram_tensor("table", [n_rows, F], mybir.dt.float32, kind="ExternalInput")
    idx_t = nc.dram_tensor("idx_t", [P, ntiles], mybir.dt.int32, kind="ExternalInput")
    out = nc.dram_tensor("out", [64 * P, F], mybir.dt.float32, kind="ExternalOutput")

    with tile.TileContext(nc) as tc:
        with (
            tc.tile_pool(name="idxp", bufs=1) as idxpool,
            tc.tile_pool(name="io", bufs=6) as pool,
        ):
            idx_tile = idxpool.tile([P, ntiles], mybir.dt.int32)
            nc.sync.dma_start(out=idx_tile[:], in_=idx_t[:, :])
            for i in range(ntiles):
                t = pool.tile([P, F], mybir.dt.float32)
                nc.gpsimd.indirect_dma_start(
                    out=t[:],
                    out_offset=None,
                    in_=table[:, :],
                    in_offset=bass.IndirectOffsetOnAxis(ap=idx_tile[:, i : i + 1], axis=0),
                )
                nc.sync.dma_start(out=out[i * P : (i + 1) * P, :], in_=t[:])
    nc.compile()
    return nc


def kernel(start, mid, end, durations, max_frames):
    start = np.asarray(start, dtype=np.float32)
    mid = np.asarray(mid, dtype=np.float32)
    end = np.asarray(end, dtype=np.float32)
    dur = np.asarray(durations).astype(np.int64)
    T = int(max_frames)
    B, N, F = start.shape

    # ---- host-side index precompute (replicates reference math) ----
    cum = np.cumsum(dur, axis=1)  # [B, N]
    total = cum[:, -1]  # [B]
    t = np.arange(T, dtype=np.int64)
    seg = np.empty((B, T), dtype=np.int64)
    for b in range(B):
        seg[b] = np.searchsorted(cum[b], t, side="right")
    seg = np.minimum(seg, N - 1)
    d = np.take_along_axis(dur, seg, axis=1)
    off = np.take_along_axis(cum, seg, axis=1) - d
    p = t[None, :] - off
    mask = t[None, :] < total[:, None]  # [B, T]
    use_start = (p == 0) & (d >= 2)
    use_end = (p == d - 1) & (d >= 2)
    # table layout per batch: rows [0,N)=start, [N,2N)=mid, [2N,3N)=end, 3N=zeros
    row = np.where(use_start, seg, np.where(use_end, seg + 2 * N, seg + N))
    idx = np.where(mask, row, 3 * N).astype(np.int32)  # [B, T]

    n_rows = 3 * N + 1
    # render only up to the longest utterance, rounded up to full 128-row tiles
    max_total = int(total.max())
    ntiles = max(1, -(-max_total // P))
    assert ntiles * P <= T

    nc = _build_program(n_rows, F, ntiles)

    zrow = np.zeros((1, F), dtype=np.float32)
    in_maps = []
    for b in range(B):
        table_b = np.concatenate([start[b], mid[b], end[b], zrow], axis=0)
        idx_b = np.ascontiguousarray(idx[b, : ntiles * P].reshape(ntiles, P).T)
        in_maps.append({"table": table_b, "idx_t": idx_b})

    res = run_bass_kernel_spmd(nc, in_maps, core_ids=list(range(B)))

    out = np.stack([res.results[b]["out"] for b in range(B)], axis=0)
    if out.shape[1] != T:  # defensive; out is built as [8192, F]
        full = np.zeros((B, T, F), dtype=np.float32)
        full[:, : out.shape[1]] = out
        out = full
    return out, mask
